# revision 15
# baseline (speedup 1.0000x reference)
"""Trainium2 Bass kernel for nn_AttentionProbe_80891414053184.

Math (reference):
    y  = relu(x @ W1.T + b1)            # (B,S,H) -> (B,S,128)
    y2 = relu(y @ W2.T + b2)            # (B,S,128)
    l  = y2 @ Wq.T + pos*pos_w  (+mask) # (B,S,8) logits
    p  = softmax(l, axis=S)
    v  = y2 @ Wv.T + bv
    out[b] = sum_{s,h} p*v + bias       # (B,1)

Strategy: sequence-parallel over 8 cores (512 positions x 4 batches = 2048
tokens per core).  Each core streams its x-shard quantized to fp8-e4m3
(half the HBM bytes of bf16; end-to-end rel-err vs the fp32 reference
~3.5e-3, HW-measured), TILE-MAJOR (tile = batch): tile t's layer-1
DoubleRow fp8 matmuls, MLP tail and softmax stats all run while tile t+1
is still streaming, so only tile 3's tail is exposed past the last HBM
byte.  Per-tile partial softmax stats (-max, Z, W) are emitted per
(seq-quarter, head) lane; the host merges 8 cores x 4 quarters with the
standard online-softmax combine and produces the (4,1) output.

fp8 scaling: W1 is pre-scaled by 64 on the host so its N(0, 1/4096)
entries land in e4m3's normal range (min normal 2^-6); the 64x is folded
back via b1*64 at the relu (relu commutes with positive scale) and W2/64
in the layer-2 weights.  x itself is N(0,1) -- quantized unscaled.

Stats packing: per tile, 4 combined q|v head-projection matmuls (one per
128-column seq quarter) land in one (128, 128) psum via tile_position
column offsets: lanes 32*qq+h hold q, lanes 32*qq+8+h hold v.  The
softmax-stats chain then runs on 128-column DVE/ACT ops (~4x shorter than
a 512-column chain), with e*v reading the v lanes through a +8-partition
shifted AP.
"""

import numpy as np

# Problem dims (hardcoded per harness contract).
B, S, H = 4, 4096, 4096
MLP, NH = 128, 8
NCORES = 8
S_SHARD = S // NCORES        # 512 seq positions per core
TOK = B * S_SHARD            # 2048 tokens per core
NT = TOK // 512              # 4 token tiles of 512 (= one batch each)
NQ = 4                       # seq quarters per tile (128 cols each)
KC2 = H // 256               # 16 double-chunks (256-deep DoubleRow contraction)
W1SCALE = 64.0               # fp8 pre-scale for W1 (power of 2, exact)

# x DMA groups, in double-chunks, per tile.  Tile 3's final group is small
# so its last-chunk matmuls gate on a 256 KB transfer, not 1 MB.
GROUPS = [[8, 8], [8, 8], [8, 8], [8, 6, 2]]

_cache = {}


def _build_nc():
    import concourse.mybir as mybir
    import concourse.tile as tile
    from concourse import bacc

    f32 = mybir.dt.float32
    f32r = mybir.dt.float32r
    fp8 = mybir.dt.float8e4
    DR = mybir.MatmulPerfMode.DoubleRow

    # Bacc (not bare Bass): its finalize() runs move_matmul_waits_to_ldweights
    # and generate_event_semaphores, which split multi-sem waits to satisfy
    # TRN2's one-wait-per-instruction encoding limit.
    nc = bacc.Bacc()
    # x, tile-major partition-contiguous: xt[t, p, c, j, n] = x feature
    # (c*256 + j*128 + p) of token (batch t, seq n) -- per partition each
    # chunk group is one contiguous DMA segment.
    xt_d = nc.dram_tensor("xt", [NT, 128, KC2, 2, 512], fp8,
                          kind="ExternalInput")
    # w1s[p, c, j, m] = 64 * W1[m, c*256 + j*128 + p]
    w1_d = nc.dram_tensor("w1s", [128, KC2, 2, MLP], fp8, kind="ExternalInput")
    # cwr: W2.T/64 f32r -- the layer-2 matmul at the fast PE rate
    cwr_d = nc.dram_tensor("cwr", [MLP, MLP], f32r, kind="ExternalInput")
    # chd: [Wq.T | Wv.T] bf16 -- the 128-col per-quarter head matmuls run at
    # 1 cycle/row (f32r would hit its 4x penalty below 256 moving cols)
    bf16 = mybir.dt.bfloat16
    chd_d = nc.dram_tensor("chd", [MLP, 2 * NH], bf16, kind="ExternalInput")
    # cw: [64*b1 | b2] per-partition scalar columns
    cw_d = nc.dram_tensor("cw", [MLP, 2], f32, kind="ExternalInput")
    # ca: per tile a (128, 128) additive-logit block (pos_w*pos + mask) on q
    # lanes, zeros on v lanes; final col = bv on v lanes.
    ca_d = nc.dram_tensor("ca", [128, NT * 128 + 1], f32, kind="ExternalInput")
    # stats: per tile 3 cols [-m | Z | W]; q lanes 32*qq+h are valid.
    st_d = nc.dram_tensor("stats", [128, NT * 3], f32, kind="ExternalOutput")

    AF = mybir.ActivationFunctionType
    AX = mybir.AxisListType
    OP = mybir.AluOpType

    with tile.TileContext(nc) as tc:
        with (
            tc.tile_pool(name="const", bufs=1) as const,
            tc.tile_pool(name="yp", bufs=2) as yp,
            tc.tile_pool(name="y2p", bufs=2) as y2p,
            tc.tile_pool(name="smallp", bufs=2) as smallp,
            tc.tile_pool(name="statsp", bufs=1) as statsp,
            tc.tile_pool(name="ps_y", bufs=2, space="PSUM") as ps_y,
            tc.tile_pool(name="ps_y2", bufs=2, space="PSUM") as ps_y2,
            tc.tile_pool(name="ps_q", bufs=2, space="PSUM") as ps_q,
            tc.tile_pool(name="ps_v", bufs=2, space="PSUM") as ps_v,
        ):
            # The full fp8 x-shard lives in SBUF (64 KB/partition): no slot
            # recycling, so the stream DMAs carry no WAR deps on the PE and
            # need no escort ops.  Issue every x DMA up front on the gpsimd
            # (SWDGE) queue; Q7 descriptor emission (~1.2 us each) stays
            # ahead of the ~3 us per-group transfer time.
            x_sb = [const.tile([128, KC2, 2, 512], fp8, name=f"x{t}")
                    for t in range(NT)]
            for t in range(NT):
                c0 = 0
                for g in GROUPS[t]:
                    nc.gpsimd.dma_start(
                        out=x_sb[t][:, c0:c0 + g, :, :],
                        in_=xt_d[t, :, c0:c0 + g, :, :])
                    c0 += g

            # Const loads on the sync (HWDGE) queue -- separate path from the
            # x stream.  w1 chunk 0 is split out so the first matmul gates on
            # a 32 KB transfer, not the full 512 KB.
            w1_sb = const.tile([128, KC2, 2, MLP], fp8)
            nc.sync.dma_start(out=w1_sb[:, 0:1, :, :], in_=w1_d[:, 0:1, :, :])
            nc.sync.dma_start(out=w1_sb[:, 1:KC2, :, :], in_=w1_d[:, 1:KC2, :, :])
            ca_sb = const.tile([128, NT * 128 + 1], f32)
            nc.sync.dma_start(out=ca_sb[:], in_=ca_d[:])
            cw_sb = const.tile([MLP, 2], f32)
            nc.sync.dma_start(out=cw_sb[:], in_=cw_d[:])
            cwr_sb = const.tile([MLP, MLP], f32r)
            nc.sync.dma_start(out=cwr_sb[:], in_=cwr_d[:])
            chd_sb = const.tile([MLP, 2 * NH], bf16)
            nc.sync.dma_start(out=chd_sb[:], in_=chd_d[:])

            stats_sb = statsp.tile([128, NT * 3], f32)

            # --- Warmup / staging: each engine observes every const-DMA lane
            # once, so steady-state instructions carry at most one new wait
            # (fewer split-events from Bacc's generate_event_semaphores).
            # Only the w1 warmup gates the k-loop; cw/ca/cwr warms run later
            # so the first real matmul waits on nothing but w1-chunk0 +
            # x-tile0-group0.
            warm_ps = ps_y2.tile([128, 512], f32, tag="y2", name="warm_ps")
            warm_pe_last = nc.tensor.matmul(warm_ps[:, 0:NH],
                                            w1_sb[:, 0, :, :],
                                            w1_sb[:, 0, :, 0:NH],
                                            start=True, stop=True,
                                            perf_mode=DR)

            first_mm = [None]

            def l1_tile(t, psum):
                # Layer 1: y_ps (128, 512) += (64*W1T)_c.T @ xT_c, DoubleRow
                # fp8 (256-deep contraction per matmul) over 16 double-chunks.
                for c in range(KC2):
                    mm = nc.tensor.matmul(
                        psum[:],
                        w1_sb[:, c, :, :],
                        x_sb[t][:, c, :, :],
                        start=(c == 0),
                        stop=(c == KC2 - 1),
                        perf_mode=DR,
                    )
                    if first_mm[0] is None:
                        first_mm[0] = mm
                        tile.add_dep_helper(mm.ins, warm_pe_last.ins,
                                            sync=False,
                                            reason="warmups before first mm")

            def tail_tile(t, psum):
                # MLP tail + per-tile softmax stats.
                y_sb = yp.tile([128, 512], f32r, tag="ysb", name=f"y_sb{t}")
                # relu on DVE (add+max) keeps ACT free for relu2/exp.  psum
                # holds 64*(x@W1.T); +64*b1 then max(.,0) gives 64*y, and
                # W2.T/64 in cwr cancels the scale at layer 2.
                nc.vector.tensor_scalar(out=y_sb[:], in0=psum[:],
                                        scalar1=cw_sb[:, 0:1],
                                        scalar2=0.0, op0=OP.add, op1=OP.max)
                y2_ps = ps_y2.tile([128, 512], f32, tag="y2", name=f"y2_ps{t}")
                nc.tensor.matmul(y2_ps[:], cwr_sb[:, 0:MLP], y_sb[:],
                                 start=True, stop=True)
                y2_sb = y2p.tile([128, 512], bf16, tag="y2sb",
                                 name=f"y2_sb{t}")
                nc.scalar.activation(out=y2_sb[:], in_=y2_ps[:], func=AF.Relu,
                                     bias=cw_sb[:, 1:2], scale=1.0)
                # q and v head projections, one pair per seq quarter, landing
                # at lanes 32*qq+h of their psums via tile_position column
                # offsets (DVE ops need partition-base-0 APs, so q and v live
                # in separate psums on the SAME lanes).
                q_ps = ps_q.tile([128, 128], f32, tag="q", name=f"q_ps{t}")
                v_ps = ps_v.tile([128, 128], f32, tag="v", name=f"v_ps{t}")
                for qq in range(NQ):
                    rhs = y2_sb[:, 128 * qq:128 * (qq + 1)]
                    nc.tensor.matmul(q_ps[32 * qq:32 * qq + NH, :],
                                     chd_sb[:, 0:NH], rhs,
                                     start=True, stop=True,
                                     tile_position=(0, 32 * qq))
                    nc.tensor.matmul(v_ps[32 * qq:32 * qq + NH, :],
                                     chd_sb[:, NH:2 * NH], rhs,
                                     start=True, stop=True,
                                     tile_position=(0, 32 * qq))
                # l = q + (pos_w*pos + mask)
                l_sb = smallp.tile([128, 128], f32, tag="l", name=f"l_sb{t}")
                nc.vector.tensor_add(out=l_sb[:], in0=q_ps[:],
                                     in1=ca_sb[:, 128 * t:128 * (t + 1)])
                # stats[:, 0] = -max_n l   (valid on q lanes)
                nc.vector.tensor_reduce(out=stats_sb[:, 3 * t:3 * t + 1],
                                        in_=l_sb[:], axis=AX.X, op=OP.max,
                                        negate=True)
                # vb = v + bv  (bv on q lanes of ca's last col)
                vb_sb = smallp.tile([128, 128], f32, tag="vb",
                                    name=f"vb_sb{t}")
                nc.vector.tensor_scalar_add(
                    out=vb_sb[:], in0=v_ps[:],
                    scalar1=ca_sb[:, NT * 128:NT * 128 + 1])
                # e = exp(l - max); stats[:, 1] = Z = sum e  (q lanes)
                e_sb = smallp.tile([128, 128], f32, tag="e", name=f"e_sb{t}")
                nc.scalar.activation(out=e_sb[:], in_=l_sb[:], func=AF.Exp,
                                     bias=stats_sb[:, 3 * t:3 * t + 1],
                                     scale=1.0,
                                     accum_out=stats_sb[:,
                                                        3 * t + 1:3 * t + 2])
                ev_sb = smallp.tile([128, 128], f32, tag="ev",
                                    name=f"ev_sb{t}")
                nc.vector.tensor_mul(out=ev_sb[:], in0=e_sb[:], in1=vb_sb[:])
                # stats[:, 2] = W = sum e*v  (q lanes)
                nc.vector.tensor_reduce(out=stats_sb[:, 3 * t + 2:3 * t + 3],
                                        in_=ev_sb[:], axis=AX.X, op=OP.add)

            psum_y = [ps_y.tile([128, 512], f32, tag="y", name=f"y_ps{t}")
                      for t in range(NT)]
            l1_tile(0, psum_y[0])
            # cw/ca/cwr/chd lane warmups (before their first consumers in
            # tile 0's tail)
            warm_ps2 = ps_y2.tile([128, 512], f32, tag="y2", name="warm_ps2")
            nc.tensor.matmul(warm_ps2[0:NH, 0:NH], cwr_sb[:, 0:NH],
                             cwr_sb[:, 0:NH], start=True, stop=True)
            nc.tensor.matmul(warm_ps2[0:2 * NH, NH:2 * NH], chd_sb[:],
                             chd_sb[:, 0:NH], start=True, stop=True)
            warm_act = const.tile([MLP, 1], f32)
            nc.scalar.copy(out=warm_act[:], in_=cw_sb[:, 1:2])
            warm_act8 = const.tile([128, 1], f32)
            nc.scalar.copy(out=warm_act8[:], in_=ca_sb[:, NT * 128:NT * 128 + 1])
            warm_dve = const.tile([128, 1], f32)
            nc.vector.tensor_copy(out=warm_dve[:], in_=ca_sb[:, 0:1])

            # PE program order: tail_t slots into PE idle gaps -- tail_2 runs
            # before L1_3 (whose x is still streaming), so after the last HBM
            # byte only tile 3's own tail remains.
            l1_tile(1, psum_y[1])
            tail_tile(0, psum_y[0])
            l1_tile(2, psum_y[2])
            tail_tile(1, psum_y[1])
            tail_tile(2, psum_y[2])
            l1_tile(3, psum_y[3])
            tail_tile(3, psum_y[3])

            nc.sync.dma_start(out=st_d[:], in_=stats_sb[:])

    nc.finalize()
    return nc


def get_nc():
    if "nc" not in _cache:
        _cache["nc"] = _build_nc()
    return _cache["nc"]


def make_core_inputs(x, mask, W1, b1, W2, b2, Wq, Wv, bv, pos_w, bias):
    """Host-side shard + transpose + fp8 quantization.  Returns list of 8
    in_maps."""
    import ml_dtypes
    e4 = ml_dtypes.float8_e4m3     # TRN FP8_EXP4: bias 7, max +-240, has inf

    # w1s[p, c, j, m] = 64 * W1[m, c*256 + j*128 + p], e4m3
    w1q = (W1.astype(np.float32) * np.float32(W1SCALE)).astype(e4)
    w1s = np.ascontiguousarray(
        w1q.reshape(MLP, KC2, 2, 128).transpose(3, 1, 2, 0))

    cwr = np.ascontiguousarray((W2.T / np.float32(W1SCALE)).astype(np.float32))
    chd = np.concatenate([Wq.T, Wv.T], axis=1).astype(ml_dtypes.bfloat16)
    cw = np.stack([b1.astype(np.float32) * np.float32(W1SCALE),
                   b2.astype(np.float32)], axis=1)  # (MLP, 2)

    pos = np.arange(S, dtype=np.float32)
    maskadd = np.where(mask == 0, np.float32(-1e9), np.float32(0.0))  # (B,S)

    in_maps = []
    for c in range(NCORES):
        sl = slice(c * S_SHARD, (c + 1) * S_SHARD)
        # xt[t, p, c2, j, n] = x[batch t, seq s0+qq*?.. wait: token col n of
        # chunk layout; feature f = c2*256 + j*128 + p, token (t, n).
        xq = x[:, sl, :].astype(e4)                    # (B, 512, H)
        xt = np.ascontiguousarray(
            xq.transpose(0, 2, 1)                      # (B, H, 512)
              .reshape(NT, KC2, 2, 128, 512)
              .transpose(0, 3, 1, 2, 4))               # (B, 128, KC2, 2, 512)
        # ca: per tile a (128, 128) block; lane 32qq+h (q): additive logit
        # for seq position qq*128+n.  Last col: bv on the same q lanes.
        ca = np.zeros((128, NT * 128 + 1), dtype=np.float32)
        addv = (pos_w.astype(np.float32)[None, :, None]
                * pos[sl][None, None, :]
                + maskadd[:, None, sl])                # (B, NH, 512)
        addv = addv.reshape(NT, NH, NQ, 128)
        for t in range(NT):
            for qq in range(NQ):
                ca[32 * qq:32 * qq + NH, 128 * t:128 * (t + 1)] = \
                    addv[t, :, qq, :]
        for qq in range(NQ):
            ca[32 * qq:32 * qq + NH, NT * 128] = bv
        in_maps.append({"xt": xt, "w1s": w1s, "cw": cw, "cwr": cwr,
                        "chd": chd, "ca": ca})
    return in_maps


def merge_stats(stats_all, bias):
    """stats_all: (NCORES, 128, NT*3); lane 32qq+h of col block 3t holds
    [-m, Z, W] for (core, quarter qq, batch t, head h) -> (B, 1) output."""
    st = np.asarray(stats_all, dtype=np.float64).reshape(NCORES, NQ, 32,
                                                         NT, 3)
    st = st[:, :, 0:NH]                   # (C, NQ, NH, NT, 3) valid q lanes
    m = -st[..., 0]                       # (C, NQ, NH, NT)
    Z = st[..., 1]
    W = st[..., 2]
    M = m.max(axis=(0, 1))                # (NH, NT)
    alpha = np.exp(m - M[None, None])
    Zg = (alpha * Z).sum(axis=(0, 1))     # (NH, NT)
    Wg = (alpha * W).sum(axis=(0, 1))
    out = (Wg / Zg).sum(axis=0)           # (NT,) = (B,)
    return (out[:, None] + np.float64(bias.reshape(1)[0])).astype(np.float32)


def kernel(x, mask, W1, b1, W2, b2, Wq, Wv, bv, pos_w, bias, _trace=False):
    from concourse.bass_utils import run_bass_kernel_spmd

    x = np.asarray(x, dtype=np.float32)
    in_maps = make_core_inputs(x, np.asarray(mask), *(np.asarray(a) for a in
                               (W1, b1, W2, b2, Wq, Wv, bv, pos_w, bias)))
    nc = get_nc()
    res = run_bass_kernel_spmd(nc, in_maps, core_ids=list(range(NCORES)),
                               trace=_trace)
    stats_all = np.stack([r["stats"] for r in res.results])  # (C, 128, NT*3)
    out = merge_stats(stats_all, np.asarray(bias))
    if _trace:
        kernel.last_result = res
    return out


# revision 19
# speedup vs baseline: 1.2315x; 1.2315x over previous
"""Trainium2 Bass kernel for nn_AttentionProbe_80891414053184.

Math (reference):
    y  = relu(x @ W1.T + b1)            # (B,S,H) -> (B,S,128)
    y2 = relu(y @ W2.T + b2)            # (B,S,128)
    l  = y2 @ Wq.T + pos*pos_w  (+mask) # (B,S,8) logits
    p  = softmax(l, axis=S)
    v  = y2 @ Wv.T + bv
    out[b] = sum_{s,h} p*v + bias       # (B,1)

Strategy: sequence-parallel over 8 cores (512 positions x 4 batches = 2048
tokens per core).  Each core streams its x-shard quantized to fp8-e4m3
(half the HBM bytes of bf16; end-to-end rel-err vs the fp32 reference
~3.5e-3, HW-measured), TILE-MAJOR (tile = batch): tile t's layer-1
DoubleRow fp8 matmuls, MLP tail and softmax stats all run while tile t+1
is still streaming, so only tile 3's tail is exposed past the last HBM
byte.  Per-tile partial softmax stats (-max, Z, W) are emitted per
(seq-quarter, head) lane; the host merges 8 cores x 4 quarters with the
standard online-softmax combine and produces the (4,1) output.

fp8 scaling: W1 is pre-scaled by 64 on the host so its N(0, 1/4096)
entries land in e4m3's normal range (min normal 2^-6); the 64x is folded
back via b1*64 at the relu (relu commutes with positive scale) and W2/64
in the layer-2 weights.  x itself is N(0,1) -- quantized unscaled.

Stats packing: per tile, 4 combined q|v head-projection matmuls (one per
128-column seq quarter) land in one (128, 128) psum via tile_position
column offsets: lanes 32*qq+h hold q, lanes 32*qq+8+h hold v.  The
softmax-stats chain then runs on 128-column DVE/ACT ops (~4x shorter than
a 512-column chain), with e*v reading the v lanes through a +8-partition
shifted AP.
"""

import numpy as np

# Problem dims (hardcoded per harness contract).
B, S, H = 4, 4096, 4096
MLP, NH = 128, 8
NCORES = 8
S_SHARD = S // NCORES        # 512 seq positions per core
TOK = B * S_SHARD            # 2048 tokens per core
NT = TOK // 512              # 4 token tiles of 512 (= one batch each)
NQ = 4                       # seq quarters per tile (128 cols each)
KC2 = H // 256               # 16 double-chunks (256-deep DoubleRow contraction)
W1SCALE = 64.0               # fp8 pre-scale for W1 (power of 2, exact)

# x DMA groups, in double-chunks, per tile.  Tile 3's final group is small
# so its last-chunk matmuls gate on a 256 KB transfer, not 1 MB.
GROUPS = [[8, 8], [8, 8], [8, 8], [8, 6, 2]]

_cache = {}


def _build_nc():
    import concourse.mybir as mybir
    import concourse.tile as tile
    from concourse import bacc

    f32 = mybir.dt.float32
    f32r = mybir.dt.float32r
    fp8 = mybir.dt.float8e4
    DR = mybir.MatmulPerfMode.DoubleRow

    # Bacc (not bare Bass): its finalize() runs move_matmul_waits_to_ldweights
    # and generate_event_semaphores, which split multi-sem waits to satisfy
    # TRN2's one-wait-per-instruction encoding limit.
    nc = bacc.Bacc()
    # x, tile-major partition-contiguous: xt[t, p, j, c, n] = x feature
    # (c*256 + j*128 + p) of token (batch t, seq n).  The two DoubleRow
    # k-tile planes (j) sit 8 KB apart per partition -- at 512 B apart the
    # double-pumped PE read drops to half rate (measured; SBUF bank
    # conflict), at 2 KB+ it runs full speed.
    xt_d = nc.dram_tensor("xt", [NT, 128, 2, KC2, 512], fp8,
                          kind="ExternalInput")
    # w1s[p, c, j, m] = 64 * W1[m, c*256 + j*128 + p]
    w1_d = nc.dram_tensor("w1s", [128, KC2, 2, MLP], fp8, kind="ExternalInput")
    # cwr: W2.T/64 f32r -- the layer-2 matmul at the fast PE rate
    cwr_d = nc.dram_tensor("cwr", [MLP, MLP], f32r, kind="ExternalInput")
    # chd: [Wq.T | Wv.T] bf16 -- the 128-col per-quarter head matmuls run at
    # 1 cycle/row (f32r would hit its 4x penalty below 256 moving cols)
    bf16 = mybir.dt.bfloat16
    chd_d = nc.dram_tensor("chd", [MLP, 2 * NH], bf16, kind="ExternalInput")
    # cw: [64*b1 | b2] per-partition scalar columns
    cw_d = nc.dram_tensor("cw", [MLP, 2], f32, kind="ExternalInput")
    # ca: per tile a (128, 128) additive-logit block (pos_w*pos + mask) on q
    # lanes, zeros on v lanes; final col = bv on v lanes.
    ca_d = nc.dram_tensor("ca", [128, NT * 128 + 1], f32, kind="ExternalInput")
    # stats: per tile 3 cols [-m | Z | W]; q lanes 32*qq+h are valid.
    st_d = nc.dram_tensor("stats", [128, NT * 3], f32, kind="ExternalOutput")

    AF = mybir.ActivationFunctionType
    AX = mybir.AxisListType
    OP = mybir.AluOpType

    with tile.TileContext(nc) as tc:
        with (
            tc.tile_pool(name="const", bufs=1) as const,
            tc.tile_pool(name="yp", bufs=2) as yp,
            tc.tile_pool(name="y2p", bufs=2) as y2p,
            tc.tile_pool(name="smallp", bufs=2) as smallp,
            tc.tile_pool(name="statsp", bufs=1) as statsp,
            tc.tile_pool(name="ps_y", bufs=2, space="PSUM") as ps_y,
            tc.tile_pool(name="ps_y2", bufs=2, space="PSUM") as ps_y2,
            tc.tile_pool(name="ps_q", bufs=2, space="PSUM") as ps_q,
            tc.tile_pool(name="ps_v", bufs=2, space="PSUM") as ps_v,
        ):
            # The full fp8 x-shard lives in SBUF (64 KB/partition): no slot
            # recycling, so the stream DMAs carry no WAR deps on the PE and
            # need no escort ops.  Issue every x DMA up front on the gpsimd
            # (SWDGE) queue; Q7 descriptor emission (~1.2 us each) stays
            # ahead of the ~3 us per-group transfer time.
            x_sb = [const.tile([128, 2, KC2, 512], fp8, name=f"x{t}")
                    for t in range(NT)]
            for t in range(NT):
                c0 = 0
                for g in GROUPS[t]:
                    nc.gpsimd.dma_start(
                        out=x_sb[t][:, :, c0:c0 + g, :],
                        in_=xt_d[t, :, :, c0:c0 + g, :])
                    c0 += g

            # Const loads on the sync (HWDGE) queue -- separate path from the
            # x stream.  w1 chunk 0 is split out so the first matmul gates on
            # a 32 KB transfer, not the full 512 KB.
            w1_sb = const.tile([128, KC2, 2, MLP], fp8)
            nc.sync.dma_start(out=w1_sb[:, 0:1, :, :], in_=w1_d[:, 0:1, :, :])
            nc.sync.dma_start(out=w1_sb[:, 1:KC2, :, :], in_=w1_d[:, 1:KC2, :, :])
            ca_sb = const.tile([128, NT * 128 + 1], f32)
            nc.sync.dma_start(out=ca_sb[:], in_=ca_d[:])
            cw_sb = const.tile([MLP, 2], f32)
            nc.sync.dma_start(out=cw_sb[:], in_=cw_d[:])
            cwr_sb = const.tile([MLP, MLP], f32r)
            nc.sync.dma_start(out=cwr_sb[:], in_=cwr_d[:])
            chd_sb = const.tile([MLP, 2 * NH], bf16)
            nc.sync.dma_start(out=chd_sb[:], in_=chd_d[:])

            stats_sb = statsp.tile([128, NT * 3], f32)

            # --- Warmup / staging: each engine observes every const-DMA lane
            # once, so steady-state instructions carry at most one new wait
            # (fewer split-events from Bacc's generate_event_semaphores).
            # Only the w1 warmup gates the k-loop; cw/ca/cwr warms run later
            # so the first real matmul waits on nothing but w1-chunk0 +
            # x-tile0-group0.
            warm_ps = ps_y2.tile([128, 512], f32, tag="y2", name="warm_ps")
            warm_pe_last = nc.tensor.matmul(warm_ps[:, 0:NH],
                                            w1_sb[:, 0, :, :],
                                            w1_sb[:, 0, :, 0:NH],
                                            start=True, stop=True,
                                            perf_mode=DR)

            first_mm = [None]

            def l1_tile(t, psum):
                # Layer 1: y_ps (128, 512) += (64*W1T)_c.T @ xT_c, DoubleRow
                # fp8 (256-deep contraction per matmul) over 16 double-chunks.
                for c in range(KC2):
                    mm = nc.tensor.matmul(
                        psum[:],
                        w1_sb[:, c, :, :],
                        x_sb[t][:, :, c, :],
                        start=(c == 0),
                        stop=(c == KC2 - 1),
                        perf_mode=DR,
                    )
                    if first_mm[0] is None:
                        first_mm[0] = mm
                        tile.add_dep_helper(mm.ins, warm_pe_last.ins,
                                            sync=False,
                                            reason="warmups before first mm")

            def tail_tile(t, psum):
                # MLP tail + per-tile softmax stats.
                y_sb = yp.tile([128, 512], f32r, tag="ysb", name=f"y_sb{t}")
                # relu on DVE (add+max) keeps ACT free for relu2/exp.  psum
                # holds 64*(x@W1.T); +64*b1 then max(.,0) gives 64*y, and
                # W2.T/64 in cwr cancels the scale at layer 2.
                nc.vector.tensor_scalar(out=y_sb[:], in0=psum[:],
                                        scalar1=cw_sb[:, 0:1],
                                        scalar2=0.0, op0=OP.add, op1=OP.max)
                y2_ps = ps_y2.tile([128, 512], f32, tag="y2", name=f"y2_ps{t}")
                nc.tensor.matmul(y2_ps[:], cwr_sb[:, 0:MLP], y_sb[:],
                                 start=True, stop=True)
                y2_sb = y2p.tile([128, 512], bf16, tag="y2sb",
                                 name=f"y2_sb{t}")
                nc.scalar.activation(out=y2_sb[:], in_=y2_ps[:], func=AF.Relu,
                                     bias=cw_sb[:, 1:2], scale=1.0)
                # q and v head projections, one pair per seq quarter, landing
                # at lanes 32*qq+h of their psums via tile_position column
                # offsets (DVE ops need partition-base-0 APs, so q and v live
                # in separate psums on the SAME lanes).
                q_ps = ps_q.tile([128, 128], f32, tag="q", name=f"q_ps{t}")
                v_ps = ps_v.tile([128, 128], f32, tag="v", name=f"v_ps{t}")
                for qq in range(NQ):
                    rhs = y2_sb[:, 128 * qq:128 * (qq + 1)]
                    nc.tensor.matmul(q_ps[32 * qq:32 * qq + NH, :],
                                     chd_sb[:, 0:NH], rhs,
                                     start=True, stop=True,
                                     tile_position=(0, 32 * qq))
                    nc.tensor.matmul(v_ps[32 * qq:32 * qq + NH, :],
                                     chd_sb[:, NH:2 * NH], rhs,
                                     start=True, stop=True,
                                     tile_position=(0, 32 * qq))
                # l = q + (pos_w*pos + mask)
                l_sb = smallp.tile([128, 128], f32, tag="l", name=f"l_sb{t}")
                nc.vector.tensor_add(out=l_sb[:], in0=q_ps[:],
                                     in1=ca_sb[:, 128 * t:128 * (t + 1)])
                # stats[:, 0] = -max_n l   (valid on q lanes)
                nc.vector.tensor_reduce(out=stats_sb[:, 3 * t:3 * t + 1],
                                        in_=l_sb[:], axis=AX.X, op=OP.max,
                                        negate=True)
                # vb = v + bv  (bv on q lanes of ca's last col)
                vb_sb = smallp.tile([128, 128], f32, tag="vb",
                                    name=f"vb_sb{t}")
                nc.vector.tensor_scalar_add(
                    out=vb_sb[:], in0=v_ps[:],
                    scalar1=ca_sb[:, NT * 128:NT * 128 + 1])
                # e = exp(l - max); stats[:, 1] = Z = sum e  (q lanes)
                e_sb = smallp.tile([128, 128], f32, tag="e", name=f"e_sb{t}")
                nc.scalar.activation(out=e_sb[:], in_=l_sb[:], func=AF.Exp,
                                     bias=stats_sb[:, 3 * t:3 * t + 1],
                                     scale=1.0,
                                     accum_out=stats_sb[:,
                                                        3 * t + 1:3 * t + 2])
                ev_sb = smallp.tile([128, 128], f32, tag="ev",
                                    name=f"ev_sb{t}")
                nc.vector.tensor_mul(out=ev_sb[:], in0=e_sb[:], in1=vb_sb[:])
                # stats[:, 2] = W = sum e*v  (q lanes)
                nc.vector.tensor_reduce(out=stats_sb[:, 3 * t + 2:3 * t + 3],
                                        in_=ev_sb[:], axis=AX.X, op=OP.add)

            psum_y = [ps_y.tile([128, 512], f32, tag="y", name=f"y_ps{t}")
                      for t in range(NT)]
            l1_tile(0, psum_y[0])
            # cw/ca/cwr/chd lane warmups (before their first consumers in
            # tile 0's tail)
            warm_ps2 = ps_y2.tile([128, 512], f32, tag="y2", name="warm_ps2")
            nc.tensor.matmul(warm_ps2[0:NH, 0:NH], cwr_sb[:, 0:NH],
                             cwr_sb[:, 0:NH], start=True, stop=True)
            nc.tensor.matmul(warm_ps2[0:2 * NH, NH:2 * NH], chd_sb[:],
                             chd_sb[:, 0:NH], start=True, stop=True)
            warm_act = const.tile([MLP, 1], f32)
            nc.scalar.copy(out=warm_act[:], in_=cw_sb[:, 1:2])
            warm_act8 = const.tile([128, 1], f32)
            nc.scalar.copy(out=warm_act8[:], in_=ca_sb[:, NT * 128:NT * 128 + 1])
            warm_dve = const.tile([128, 1], f32)
            nc.vector.tensor_copy(out=warm_dve[:], in_=ca_sb[:, 0:1])

            # PE program order: tail_t slots into PE idle gaps -- tail_2 runs
            # before L1_3 (whose x is still streaming), so after the last HBM
            # byte only tile 3's own tail remains.
            l1_tile(1, psum_y[1])
            tail_tile(0, psum_y[0])
            l1_tile(2, psum_y[2])
            tail_tile(1, psum_y[1])
            tail_tile(2, psum_y[2])
            l1_tile(3, psum_y[3])
            tail_tile(3, psum_y[3])

            nc.sync.dma_start(out=st_d[:], in_=stats_sb[:])

    nc.finalize()
    return nc


def get_nc():
    if "nc" not in _cache:
        _cache["nc"] = _build_nc()
    return _cache["nc"]


def make_core_inputs(x, mask, W1, b1, W2, b2, Wq, Wv, bv, pos_w, bias):
    """Host-side shard + transpose + fp8 quantization.  Returns list of 8
    in_maps."""
    import ml_dtypes
    e4 = ml_dtypes.float8_e4m3     # TRN FP8_EXP4: bias 7, max +-240, has inf

    # w1s[p, c, j, m] = 64 * W1[m, c*256 + j*128 + p], e4m3
    w1q = (W1.astype(np.float32) * np.float32(W1SCALE)).astype(e4)
    w1s = np.ascontiguousarray(
        w1q.reshape(MLP, KC2, 2, 128).transpose(3, 1, 2, 0))

    cwr = np.ascontiguousarray((W2.T / np.float32(W1SCALE)).astype(np.float32))
    chd = np.concatenate([Wq.T, Wv.T], axis=1).astype(ml_dtypes.bfloat16)
    cw = np.stack([b1.astype(np.float32) * np.float32(W1SCALE),
                   b2.astype(np.float32)], axis=1)  # (MLP, 2)

    pos = np.arange(S, dtype=np.float32)
    maskadd = np.where(mask == 0, np.float32(-1e9), np.float32(0.0))  # (B,S)

    in_maps = []
    for c in range(NCORES):
        sl = slice(c * S_SHARD, (c + 1) * S_SHARD)
        # xt[t, p, j, c2, n]: feature f = c2*256 + j*128 + p, token (t, n)
        xq = x[:, sl, :].astype(e4)                    # (B, 512, H)
        xt = np.ascontiguousarray(
            xq.transpose(0, 2, 1)                      # (B, H, 512)
              .reshape(NT, KC2, 2, 128, 512)
              .transpose(0, 3, 2, 1, 4))               # (B, 128, 2, KC2, 512)
        # ca: per tile a (128, 128) block; lane 32qq+h (q): additive logit
        # for seq position qq*128+n.  Last col: bv on the same q lanes.
        ca = np.zeros((128, NT * 128 + 1), dtype=np.float32)
        addv = (pos_w.astype(np.float32)[None, :, None]
                * pos[sl][None, None, :]
                + maskadd[:, None, sl])                # (B, NH, 512)
        addv = addv.reshape(NT, NH, NQ, 128)
        for t in range(NT):
            for qq in range(NQ):
                ca[32 * qq:32 * qq + NH, 128 * t:128 * (t + 1)] = \
                    addv[t, :, qq, :]
        for qq in range(NQ):
            ca[32 * qq:32 * qq + NH, NT * 128] = bv
        in_maps.append({"xt": xt, "w1s": w1s, "cw": cw, "cwr": cwr,
                        "chd": chd, "ca": ca})
    return in_maps


def merge_stats(stats_all, bias):
    """stats_all: (NCORES, 128, NT*3); lane 32qq+h of col block 3t holds
    [-m, Z, W] for (core, quarter qq, batch t, head h) -> (B, 1) output."""
    st = np.asarray(stats_all, dtype=np.float64).reshape(NCORES, NQ, 32,
                                                         NT, 3)
    st = st[:, :, 0:NH]                   # (C, NQ, NH, NT, 3) valid q lanes
    m = -st[..., 0]                       # (C, NQ, NH, NT)
    Z = st[..., 1]
    W = st[..., 2]
    M = m.max(axis=(0, 1))                # (NH, NT)
    alpha = np.exp(m - M[None, None])
    Zg = (alpha * Z).sum(axis=(0, 1))     # (NH, NT)
    Wg = (alpha * W).sum(axis=(0, 1))
    out = (Wg / Zg).sum(axis=0)           # (NT,) = (B,)
    return (out[:, None] + np.float64(bias.reshape(1)[0])).astype(np.float32)


def kernel(x, mask, W1, b1, W2, b2, Wq, Wv, bv, pos_w, bias, _trace=False):
    from concourse.bass_utils import run_bass_kernel_spmd

    x = np.asarray(x, dtype=np.float32)
    in_maps = make_core_inputs(x, np.asarray(mask), *(np.asarray(a) for a in
                               (W1, b1, W2, b2, Wq, Wv, bv, pos_w, bias)))
    nc = get_nc()
    res = run_bass_kernel_spmd(nc, in_maps, core_ids=list(range(NCORES)),
                               trace=_trace)
    stats_all = np.stack([r["stats"] for r in res.results])  # (C, 128, NT*3)
    out = merge_stats(stats_all, np.asarray(bias))
    if _trace:
        kernel.last_result = res
    return out
